# revision 36
# baseline (speedup 1.0000x reference)
"""Trainium2 Bass kernel for NeuronToSpatialGrid.

reference: w[p,n] = exp(-|c_p - x_n|^2 / 0.02); w /= sum_n w + 1e-8;
           out[b,e,gx,gy] = sum_n w[p,n] * F[n,e],  p = gx*64+gy.

Strategy (8 cores = 4 batches x 2 grid-halves of 2048 points = 32 gx
x 64 gy per core):

  The Gaussian separates: w[p,n] = u[gx,n] * v[gy,n], so the weight
  denominator den[gx,gy] = sum_n u[gx,n] v[gy,n] is a tiny rank-4096
  contraction instead of an elementwise reduction of the full 8.4M
  weight matrix.

  prologue (separable den -> ln fold):
    uvT[n, 128-col block] = exponents for [u cols 0:32 | v cols 32:96]
    per n-block via K=14 bf16 matmuls (x^2/cx^2 terms folded in-matmul
    so the Exp needs no per-partition bias) -> four [128,1024] ACT Exp
    instrs -> den[32,64] accumulated over 32 blocks on PE (F=64
    matmuls) -> ACT Ln -> DVE scales by -1/50 and 2-splits to bf16 ->
    flattened to [1,2048] rows via a DRAM bounce (tiny SBUF->SBUF DMAs
    would serialize as ~600ns DIRECT2D ops on the Sync engine) and
    written into crdr rows 15,16 (and 47,48 for the second PE band).

  main loop (64 windows = 4 j-tiles x 16 block-pairs):
    s1[n, 1024] = 50-scaled exponent INCLUDING -|x_n|^2 (rows 17,18)
    and -ln(den_p)/50 (rows 15,16) via two K=19 bf16 matmuls in
    separate PE row bands (tile_position (0,0)/(32,0), concurrent)
    writing one 2-bank PSUM tile; ONE ACT Exp [128,1024] -> wt bf16
    (already normalized!); four bf16 e-matmuls accumulate out[e,p]
    in PSUM over the 32 n-blocks.  j-epilogue is just two PSUM->SBUF
    copies (ACT + DVE) + DMA out: no reciprocal, no broadcast, no
    elementwise den work anywhere.

  head overlap: the prologue borrows s1-/e0-tagged PSUM ring slots
    (no pool-release barrier) and j=0 runs UNNORMALIZED concurrently
    with it (crdr rows 15,16 still zero); the ln rows land via WAR-
    ordered DMAs after j=0's last pack read and before j=1's first
    pack is emitted, and j=0 is fixed up in its epilogue with a K=1
    1/den broadcast matmul into a spare s1 slot (classic end-norm,
    rec bounced through DRAM as a [1,2048] f32 row).

  Perf notes (measured):
  - every dma_start costs ~650ns of SERIAL issue time on its engine's
    queue; feat is 4 big DMAs on the idle GpSimd queue, uvp/uvc go
    first so their transfers aren't queued behind the 2MB of feat.
  - feat and wt are bf16 (matmul dtypes must match; unbiased rounding
    -> ~2.7e-3 rel err vs the 2e-2 gate).  e-mms stream 512 cols at
    1 col/cycle; PE ~1.29us/window and ACT ~1.11us/window are the
    co-rooflines (~122us total vs 155us for the pre-separable
    baseline).
"""

import os
import numpy as np
import ml_dtypes

import concourse.bass as bass
import concourse.tile as tile
from concourse import bacc, mybir, bass_utils

BF16 = ml_dtypes.bfloat16
B, N, E, G = 4, 4096, 256, 64
P = G * G
HALF = P // 2          # grid points per core
GXH = 32               # gx columns per core (= HALF // G)
N_CORES = 8
NB = N // 128          # 32 n-blocks
NW = NB // 2           # 16 packed windows (2 blocks each) per p-tile
PJ = HALF // 512       # 4 p-tiles per core
NWIN = PJ * NW         # 64 banded windows
SIGMA2 = 2.0 * 0.1 ** 2
SCALE = 1.0 / SIGMA2   # 50.0
KUV = 14               # prologue matmul contraction rows

_CACHE = {}
LAST_EXEC_NS = None
LAST_RESULTS = None


def _split3(v):
    t1 = v.astype(BF16)
    r1 = v - t1.astype(np.float64)
    t2 = r1.astype(BF16)
    r2 = r1 - t2.astype(np.float64)
    t3 = r2.astype(BF16)
    return t1, t2, t3


def _split2(v):
    t1 = v.astype(BF16)
    t2 = (v - t1.astype(np.float64)).astype(BF16)
    return t1, t2


def _build(reps=1):
    if reps in _CACHE:
        return _CACHE[reps]
    f32 = mybir.dt.float32
    f32r = mybir.dt.float32r
    bf16 = mybir.dt.bfloat16

    nc = bacc.Bacc("TRN2", target_bir_lowering=False, debug=False,
                   enable_asserts=False, num_devices=N_CORES)

    feat_d = nc.dram_tensor("feat", [N, E], bf16, kind="ExternalInput").ap()
    posp_d = nc.dram_tensor("posp", [64, NW * 128], bf16,
                            kind="ExternalInput").ap()
    crdr_d = nc.dram_tensor("crdr", [64, HALF], bf16,
                            kind="ExternalInput").ap()
    uvp_d = nc.dram_tensor("uvprep", [KUV, N], bf16,
                           kind="ExternalInput").ap()
    uvc_d = nc.dram_tensor("uvcrd", [KUV, 128], bf16,
                           kind="ExternalInput").ap()
    lns_d = nc.dram_tensor("lns", [GXH, 2 * G], bf16, kind="ExternalInput").ap()
    oner_d = nc.dram_tensor("ones_row", [1, 128], f32r,
                            kind="ExternalInput").ap()
    recs_d = nc.dram_tensor("recs", [GXH, G], f32r,
                            kind="ExternalInput").ap()
    out_d = nc.dram_tensor("out", [E, HALF], f32, kind="ExternalOutput").ap()

    with tile.TileContext(nc) as tc:
        from contextlib import ExitStack
        with ExitStack() as ctx:
            const = ctx.enter_context(tc.tile_pool(name="const", bufs=1))
            featp = ctx.enter_context(tc.tile_pool(name="feat", bufs=1))

            posp_sb = const.tile([64, NW * 128], bf16)
            crdr_sb = const.tile([64, HALF], bf16)
            uvp_sb = const.tile([KUV, N], bf16)
            uvc_sb = const.tile([KUV, 128], bf16)
            uvt_sb = const.tile([128, NB * 128], f32r)
            lnt = const.tile([GXH, G], f32)
            oner_sb = const.tile([1, 128], f32r)
            rec_sb = const.tile([GXH, G], f32r)
            recrow = const.tile([1, HALF], f32r)
            l12 = const.tile([GXH, 2 * G], bf16)
            l1f = const.tile([GXH, G], f32)
            # warm up the ACT Exp function table before the first real Exp
            warm = const.tile([1, 8], f32)
            warm2 = const.tile([1, 8], f32)
            nc.vector.memset(warm[:], 0.0)
            nc.scalar.activation(warm2[:], warm[:],
                                 mybir.ActivationFunctionType.Exp)
            nc.gpsimd.dma_start(uvp_sb[:], uvp_d[:])
            nc.gpsimd.dma_start(uvc_sb[:], uvc_d[:])
            nc.sync.dma_start(posp_sb[:], posp_d[:])
            nc.sync.dma_start(crdr_sb[:], crdr_d[:])
            nc.sync.dma_start(oner_sb[:], oner_d[:])

            feat_sb = featp.tile([128, NB * E], bf16)
            # 4 big DMAs issued from the idle GpSimd queue: every
            # dma_start costs ~650ns of serial issue time on its engine
            for c in range(4):
                blk0 = c * 8
                src_ap = feat_d[blk0 * 128:(blk0 + 8) * 128, :].rearrange(
                    "(b p) e -> p b e", p=128)
                dst_ap = feat_sb[:, blk0 * E:(blk0 + 8) * E].rearrange(
                    "p (b e) -> p b e", b=8)
                nc.gpsimd.dma_start(dst_ap, src_ap)

            # ---- main pipeline pools (created before the prologue:
            # the prologue borrows s1-/e0-tagged PSUM slots so there is
            # no pool-release barrier and j=0 can start immediately) ----
            wtp = ctx.enter_context(tc.tile_pool(name="wt", bufs=12))
            outp = ctx.enter_context(tc.tile_pool(name="outsb", bufs=4))
            ps1 = ctx.enter_context(tc.tile_pool(name="ps1", bufs=1,
                                                 space="PSUM"))
            pse = ctx.enter_context(tc.tile_pool(name="pse", bufs=2,
                                                 space="PSUM"))

            # ---- prologue: separable den ----
            # uv_ps holds 8 n-blocks of [128, 128] exponent columns
            # (u: 0:32, v: 32:96, zero: 96:128) per quarter.
            if True:
                den_t = pse.tile([128, 512], f32, name="e0")
                den_ps = den_t[0:GXH, 0:G]

                def uvq(qtr):
                    uv_ps = ps1.tile([128, 1024], f32, name="s1",
                                     bufs=2)
                    for k in range(8):
                        blk = qtr * 8 + k
                        nc.tensor.matmul(
                            uv_ps[:, k * 128:(k + 1) * 128],
                            uvp_sb[:, blk * 128:(blk + 1) * 128],
                            uvc_sb[:],
                            start=True, stop=True)
                    nc.scalar.activation(
                        uvt_sb[:, qtr * 1024:(qtr + 1) * 1024],
                        uv_ps[:],
                        mybir.ActivationFunctionType.Exp, scale=SCALE)

                def denq(qtr):
                    for k in range(8):
                        blk = qtr * 8 + k
                        nc.tensor.matmul(
                            den_ps,
                            uvt_sb[:, blk * 128:blk * 128 + GXH],
                            uvt_sb[:, blk * 128 + GXH:blk * 128 + GXH + G],
                            start=(blk == 0), stop=(blk == NB - 1))

                # uvmms for qtr+1 are emitted before den mms for qtr so
                # the PE keeps feeding the ACT exp ladder
                uvq(0)
                uvq(1)
                denq(0)
                uvq(2)
                denq(1)
                uvq(3)
                denq(2)
                denq(3)
                nc.scalar.activation(lnt[:], den_ps,
                                     mybir.ActivationFunctionType.Ln)
                with nc.allow_low_precision(reason="f32r bit-identical"):
                    nc.vector.reciprocal(rec_sb[:], den_ps)
                # Ln-dependent dummy Exp: forces the exp-table reload
                # to happen HERE (overlapping the flatten chain below)
                # instead of right before the first main-loop Exp.  The
                # Ln data dep keeps the scheduler from hoisting it.
                warm4 = const.tile([1, 8], f32)
                nc.scalar.activation(warm4[:], lnt[0:1, 0:8],
                                     mybir.ActivationFunctionType.Exp)
            # crdr rows 15,16 (and 47,48) <- bf16 2-split of -ln(den)/50
            nc.vector.tensor_scalar_mul(lnt[:], lnt[:], -1.0 / SCALE)
            nc.vector.tensor_copy(l12[:, 0:G], lnt[:])
            with nc.allow_low_precision(reason="2-term bf16 split"):
                nc.vector.tensor_sub(l12[:, G:2 * G], lnt[:], l12[:, 0:G])
            # flatten [32,64] -> [1,2048] via a DRAM bounce (tiny
            # SBUF->SBUF DMAs would serialize as ~600ns DIRECT2D ops);
            # the two readbacks go on different queues to run in parallel
            nc.sync.dma_start(lns_d[:], l12[:])
            nc.sync.dma_start(recs_d[:], rec_sb[:])
            recflat = recs_d.rearrange("a b -> (a b)").unsqueeze(0)
            nc.sync.dma_start(recrow[0:1, :], recflat)
            rows = lns_d.rearrange("a (r b) -> r a b", r=2)
            dst1 = crdr_sb[15:17, :].rearrange("r (a b) -> r a b", a=GXH)
            dst2 = crdr_sb[47:49, :].rearrange("r (a b) -> r a b", a=GXH)

            def emit_ln_rows():
                # deferred: j=0 runs with rows 15,16 still zero
                # (unnormalized weights, fixed up via 1/den broadcast);
                # these writes are WAR-ordered after all j=0 pack reads
                nc.sync.dma_start(dst1, rows)
                nc.gpsimd.dma_start(dst2, rows)

            pools = dict(wtp=wtp, outp=outp,
                         ps1=ps1, pse=pse,
                         feat_sb=feat_sb, posp_sb=posp_sb, crdr_sb=crdr_sb,
                         oner_sb=oner_sb, recrow=recrow,
                         emit_ln_rows=emit_ln_rows,
                         out_d=out_d)
            if reps == 1:
                _emit(nc, pools)
            else:
                with tc.For_i(0, reps, 1):
                    _emit(nc, pools)

    nc.compile()
    _CACHE[reps] = nc
    return nc


def _emit(nc, pools):
    f32 = mybir.dt.float32
    f32r = mybir.dt.float32r
    bf16 = mybir.dt.bfloat16
    wtp, outp = pools["wtp"], pools["outp"]
    ps1, pse = pools["ps1"], pools["pse"]
    feat_sb, posp_sb, crdr_sb = (pools["feat_sb"], pools["posp_sb"],
                                 pools["crdr_sb"])
    oner_sb, recrow = pools["oner_sb"], pools["recrow"]
    emit_ln_rows = pools["emit_ln_rows"]
    out_d = pools["out_d"]

    s1_store = {}

    def pack(idx):
        j, g = divmod(idx, NW)
        # both bands write one 2-bank tile: band 0 -> cols 0:512,
        # band 1 -> cols 512:1024 (each range is exactly one bank, so
        # the start=True whole-bank clear is safe)
        s1 = ps1.tile([128, 1024], f32, name="s1", bufs=2)
        for bnd in range(2):
            r0 = 32 * bnd
            nc.tensor.matmul(s1[:, bnd * 512:(bnd + 1) * 512],
                             posp_sb[r0:r0 + 19, g * 128:(g + 1) * 128],
                             crdr_sb[r0:r0 + 19, j * 512:(j + 1) * 512],
                             start=True, stop=True, tile_position=(r0, 0))
        s1_store[idx] = s1

    pack(0)
    pack(1)

    e0 = e1 = None
    for idx in range(NWIN):
        j, g = divmod(idx, NW)
        if idx == NW - 2:
            # all j=0 packs (0..15) are emitted; ln rows land now,
            # WAR-ordered after every j=0 read of the still-zero rows
            # and BEFORE pack(16) = j=1's first window is emitted
            emit_ln_rows()
        if g == 0:
            e0 = pse.tile([128, 512], f32)
            e1 = pse.tile([128, 512], f32)
        s1 = s1_store.pop(idx)
        wt = wtp.tile([128, 1024], bf16)
        nc.scalar.activation(wt[:], s1[:],
                             mybir.ActivationFunctionType.Exp, scale=SCALE)
        for bnd in range(2):
            i = 2 * g + bnd
            st, sp = (i == 0), (i == NB - 1)
            wts = wt[:, bnd * 512:(bnd + 1) * 512]
            nc.tensor.matmul(e0[:], feat_sb[:, i * E:i * E + 128],
                             wts, start=st, stop=sp)
            nc.tensor.matmul(e1[:], feat_sb[:, i * E + 128:(i + 1) * E],
                             wts, start=st, stop=sp)
            if bnd == 0 and idx + 2 < NWIN:
                pack(idx + 2)
        if idx == NW + 1:
            # deferred j=0 fixup: emitted two windows into j=1 so the
            # s1-slot borrow and DVE muls don't pile onto the j0->j1
            # ring hand-off (j0's e0/e1 PSUM slots stay live until j=2)
            e0p, e1p = pend_j0
            o0 = outp.tile([128, 512], f32, name="o0", bufs=2)
            o1 = outp.tile([128, 512], f32, name="o1", bufs=2)
            bc_t = ps1.tile([128, 1024], f32, name="s1", bufs=2)
            nc.tensor.matmul(bc_t[:, 0:512], oner_sb[:],
                             recrow[0:1, 0:512],
                             start=True, stop=True)
            bc_sb = outp.tile([128, 512], f32r, name="bcsb", bufs=1)
            with nc.allow_low_precision(reason="f32r bit-identical"):
                nc.vector.tensor_copy(bc_sb[:], bc_t[:, 0:512])
                nc.vector.tensor_mul(o0[:], e0p[:], bc_sb[:])
                nc.vector.tensor_mul(o1[:], e1p[:], bc_sb[:])
            nc.gpsimd.dma_start(out_d[0:128, 0:512], o0[:])
            nc.gpsimd.dma_start(out_d[128:256, 0:512], o1[:])
        if g == NW - 1:
            if j == 0:
                pend_j0 = (e0, e1)
            else:
                # e0/e1 already normalized (ln(den) folded into s1)
                o0 = outp.tile([128, 512], f32, name="o0", bufs=2)
                o1 = outp.tile([128, 512], f32, name="o1", bufs=2)
                nc.scalar.copy(o0[:], e0[:])
                nc.vector.tensor_copy(o1[:], e1[:])
                nc.gpsimd.dma_start(out_d[0:128, j * 512:(j + 1) * 512],
                                    o0[:])
                nc.gpsimd.dma_start(out_d[128:256, j * 512:(j + 1) * 512],
                                    o1[:])


def _host_prep(neuron_features, positions):
    """Per-core input maps. Core c: batch c//2, grid half c%2."""
    lin = np.linspace(0.0, 1.0, G).astype(np.float32)
    gx, gy = np.meshgrid(lin, lin, indexing="ij")
    coords = np.stack([gx.ravel(), gy.ravel()], axis=-1).astype(np.float64)

    crdr_halves, uvcrd_halves = [], []
    for h in range(2):
        c = coords[h * HALF:(h + 1) * HALF]
        cx1, cx2, cx3 = _split3(2.0 * c[:, 0])
        cy1, cy2, cy3 = _split3(2.0 * c[:, 1])
        cn1, cn2, cn3 = _split3(c[:, 0] ** 2 + c[:, 1] ** 2)
        rows = [cx1, cx2, cx1, cx2, cx3, cx1,
                cy1, cy2, cy1, cy2, cy3, cy1,
                -cn1, -cn2, -cn3]
        crd15 = np.stack(rows, axis=0).astype(BF16)
        crd_rep = np.zeros((64, HALF), dtype=BF16)
        crd_rep[0:15] = crd15
        crd_rep[32:47] = crd15
        crd_rep[17:19] = 1.0   # pairs with -|x|^2 split rows in posp
        crd_rep[49:51] = 1.0
        crdr_halves.append(crd_rep)

        # prologue rhs: u cols = this half's 32 gx values, v cols = 64 gy
        ux = 2.0 * lin[h * GXH:(h + 1) * GXH].astype(np.float64)
        vy = 2.0 * lin.astype(np.float64)
        uxh, uxl = _split2(ux)
        vyh, vyl = _split2(vy)
        mux_h, mux_l = _split2(-(ux / 2.0) ** 2)
        mvy_h, mvy_l = _split2(-(vy / 2.0) ** 2)
        uvcrd = np.zeros((KUV, 128), dtype=BF16)
        uvcrd[0, 0:GXH] = uxh
        uvcrd[1, 0:GXH] = uxh
        uvcrd[2, 0:GXH] = uxh
        uvcrd[3, 0:GXH] = uxl
        uvcrd[4, 0:GXH] = -1.0
        uvcrd[5, 0:GXH] = -1.0
        uvcrd[6, GXH:GXH + G] = vyh
        uvcrd[7, GXH:GXH + G] = vyh
        uvcrd[8, GXH:GXH + G] = vyh
        uvcrd[9, GXH:GXH + G] = vyl
        uvcrd[10, GXH:GXH + G] = -1.0
        uvcrd[11, GXH:GXH + G] = -1.0
        uvcrd[12, 0:GXH] = mux_h
        uvcrd[12, GXH:GXH + G] = mvy_h
        uvcrd[13, 0:GXH] = mux_l
        uvcrd[13, GXH:GXH + G] = mvy_l
        uvcrd_halves.append(uvcrd)

    posp_b, uvprep_b = [], []
    for b in range(B):
        x = positions[b, :, 0].astype(np.float64)
        y = positions[b, :, 1].astype(np.float64)
        x1, x2, x3 = _split3(x)
        y1, y2, y3 = _split3(y)
        one = np.ones(N, dtype=BF16)
        rows15 = np.stack([x1, x1, x2, x2, x1, x3,
                           y1, y1, y2, y2, y1, y3,
                           one, one, one], axis=0).astype(BF16)
        pos_pack = np.zeros((64, NW * 128), dtype=BF16)
        for g in range(NW):
            pos_pack[0:15, g * 128:(g + 1) * 128] = \
                rows15[:, (2 * g) * 128:(2 * g + 1) * 128]
            pos_pack[32:47, g * 128:(g + 1) * 128] = \
                rows15[:, (2 * g + 1) * 128:(2 * g + 2) * 128]
        pos_pack[15:17] = 1.0
        pos_pack[47:49] = 1.0
        nsq = x * x + y * y
        q1 = (-nsq).astype(BF16)
        q2 = (-nsq - q1.astype(np.float64)).astype(BF16)
        for g in range(NW):
            pos_pack[17:19, g * 128:(g + 1) * 128] = np.stack(
                [q1, q2])[:, (2 * g) * 128:(2 * g + 1) * 128]
            pos_pack[49:51, g * 128:(g + 1) * 128] = np.stack(
                [q1, q2])[:, (2 * g + 1) * 128:(2 * g + 2) * 128]
        posp_b.append(pos_pack)
        xx1, xx2 = _split2(x * x)
        yy1, yy2 = _split2(y * y)
        uvprep_b.append(np.stack([x1, x2, x3, x1, xx1, xx2,
                                  y1, y2, y3, y1, yy1, yy2,
                                  one, one], axis=0).astype(BF16))

    in_maps = []
    for c in range(N_CORES):
        b, h = divmod(c, 2)
        in_maps.append({
            "feat": np.ascontiguousarray(neuron_features[b]).astype(BF16),
            "posp": posp_b[b],
            "crdr": crdr_halves[h],
            "uvprep": uvprep_b[b],
            "uvcrd": uvcrd_halves[h],
            "lns": np.zeros((GXH, 2 * G), dtype=BF16),
            "ones_row": np.ones((1, 128), np.float32),
            "recs": np.zeros((GXH, G), np.float32),
        })
    return in_maps


def kernel(neuron_features, positions):
    global LAST_EXEC_NS, LAST_RESULTS
    nf = np.ascontiguousarray(np.asarray(neuron_features, dtype=np.float32))
    pos = np.ascontiguousarray(np.asarray(positions, dtype=np.float32))
    nc = _build()
    in_maps = _host_prep(nf, pos)
    trace = bool(int(os.environ.get("KERNEL_TRACE", "0")))
    res = bass_utils.run_bass_kernel_spmd(nc, in_maps,
                                          core_ids=list(range(N_CORES)),
                                          trace=trace)
    LAST_RESULTS = res
    LAST_EXEC_NS = getattr(res, "exec_time_ns", None)
    full = np.empty((B, E, P), np.float32)
    for c in range(N_CORES):
        b, h = divmod(c, 2)
        full[b, :, h * HALF:(h + 1) * HALF] = res.results[c]["out"]
    return full.reshape(B, E, G, G)
